# revision 27
# baseline (speedup 1.0000x reference)
"""AFT-Full attention on 8 TRN2 NeuronCores (Bass/Tile, no collectives).

Reference math (B=2, TQ=TKV=512, DIM=512, HID=128, BDIM=128):
    qh  = q @ qW_w.T + qW_b
    k   = kv @ kW_w.T + kW_b
    v   = kv @ vW_w.T + vW_b
    wb  = w_bias_u @ w_bias_v                       # (TQ, TKV)
    A   = exp(k[:,None] + wb[None,:,:,None])        # (B,TQ,TKV,HID)
    out = sigmoid(qh) * (sum_s A*v / sum_s A)

Factorization: exp(k + wb) = exp(k) * exp(wb) collapses the giant A
intermediate into plain matmuls:
    num[t,h] = sum_s exp(wb[t,s]) * (exp(k0[s,h]) * v0[s,h])
    den[t,h] = sum_s exp(wb[t,s]) *  exp(k0[s,h])
The k-projection bias cancels exactly in num/den; the v bias is a pure
per-h additive term:  out = sigmoid(qh) * (num0/den0 + vW_b).

Sharding: the 1024 flattened (b, t) query rows split into 8 blocks of 128 —
core i handles batch b=i//4, queries t in [128*(i%4), 128*(i%4)+128).
Each core only needs kv[b], so no collectives (their ~7us latency floor
exceeds this whole kernel).

Measured-overhead model (trace-verified): exec_time spans from the
framework's const-pool memsets to the very end of the NEFF program, which
includes a fixed ~8us postamble (254 per-semaphore zeroing instructions).
Only the body between those is kernel-controllable, so the body is built
around the DMA stream as the single critical path:

    sync-ring order:  s0[wbv|uT fp8] s1[kvWT] s2[kv half0] Q2 Q3 s4[qW|qT]
    (~1.12 MB; first-needed-first, and the q slab LAST because its trailing
    chain -- 4 matmuls + one ACT exp -> epilogue -- is shorter than the kv
    chain proj->exp->mul->num that would trail Q3.)

Every DMA-gated compute group lands mid-stream: wb matmuls after s0 (fp8:
w_bias_{u,v} are ~N(0,0.02); host pre-scales by 16 so e4m3 quantization
contributes <1e-4 to wb -- CPU-verified rel err 3.890e-3 vs 3.889e-3 bf16),
projections per kv chunk as it arrives, den/num accumulations interleaved
so only sc3's exp/mul/num trail the last kv quarter.  Dummy warmup/filler
matmuls (128-col) keep the PE HAM clock at 2.4GHz across DMA-wait gaps.

dtype strategy: slabs are packed bf16 HOST-SIDE (wb operands fp8), halving
the DMA stream; all matmuls run native bf16/fp8, PSUM stays f32; the
result is stored/DMA'd bf16 and upcast on host (~4e-3 rel err vs the 2e-2
gate).  Full fp8 kv was tried and REJECTED: num is a random-sign sum, so
per-element ek/v quantization errors survive at full strength (4.4e-2).
Bias columns keep exact f32 bits as bf16 column pairs, bitcast back on
device.  When qW_b/vW_b are all-zero (true for this problem's
setup_inputs), a fast epilogue variant drops the bias terms; a general
variant is built lazily otherwise.

Engine choreography: sigmoid(qh) is computed as 1/(1+exp(-qh)) so ACT only
ever loads the EXP table; ek/ekv for the paired half0 use one strided ACT
exp + one DVE mul; the epilogue is 3 DVE ops (STT, fast reciprocal, mul)
with the output ring pre-warmed by a tiny primer DMA gated on ekv[sc2].
"""

import numpy as np
from ml_dtypes import bfloat16 as np_bf16
from ml_dtypes import float8_e4m3 as np_fp8

import concourse.bass as bass
import concourse.mybir as mybir
import concourse.tile as tile
from concourse import bacc
from concourse.bass_utils import run_bass_kernel_spmd

B, TQ, TKV, DIM, HID, BDIM = 2, 512, 512, 512, 128, 128
N_CORES = 8
R = (B * TQ) // N_CORES  # 128 query rows per core
P = 128
DC = DIM // P  # 4 contraction chunks for d
SC = TKV // P  # 4 contraction chunks for s
F32 = mybir.dt.float32
BF16 = mybir.dt.bfloat16
FP8 = mybir.dt.float8e4
SWB = 16.0  # host pre-scale for w_bias_u/v (fp8 range); undone in ACT scale
ACT = mybir.ActivationFunctionType
N_WARMUP = 30  # one CONTIGUOUS ~3.2us warmup run: the HAM clock unthrottles
FILL_WB = 0    # only after a fully-busy 3.4us window, so the block must not
FILL_QH = 0    # be broken by data-gated stalls; once warm, gaps are harmless

S0 = TKV + R  # 640 fp8 cols: wbv | uT
S1 = DC * 2 * HID  # 1024: kvWT
S2 = DC * (TKV // 2)  # 1024: kv s-half 0 (half layout, 2KB lines)
S3 = DC * (TKV // 2)  # 1024: kv s-half 1 (quarter layout, Q2|Q3)
S4 = DC * HID + DC * R + 4  # 1028: qWT | qT | -qb | vb


def _build(zero_bias=True):
    nc = bacc.Bacc(None)
    s0 = nc.declare_dram_parameter("s0", [P, S0], FP8, isOutput=False)
    s1 = nc.declare_dram_parameter("s1", [P, S1], BF16, isOutput=False)
    s2 = nc.declare_dram_parameter("s2", [P, S2], BF16, isOutput=False)
    s3 = nc.declare_dram_parameter("s3", [P, S3], BF16, isOutput=False)
    s4 = nc.declare_dram_parameter("s4", [P, S4], BF16, isOutput=False)
    out = nc.declare_dram_parameter("out", [HID, R], BF16, isOutput=True)

    with tile.TileContext(nc) as tc:
        with (
            tc.tile_pool(name="persist", bufs=1) as persist,
            tc.tile_pool(name="psumw", bufs=1, space="PSUM") as psumw,
            tc.tile_pool(name="psumk", bufs=2, space="PSUM") as psumk,
            tc.tile_pool(name="psum1", bufs=1, space="PSUM") as psum1,
        ):
            # ---- slab DMAs on BOTH HWDGE rings (sync + scalar): each ring
            # issues one DMA per ~0.65us, so splitting 6 transfers across two
            # rings doubles the stream's front-end issue rate.  The 16 SDMA
            # engines drain both rings round-robin per packet, so byte-fair
            # interleave preserves the completion order s0 s1 s2 Q2 Q3 s4
            # (sync carries 605KB, scalar 518KB -> s4 still lands last).
            m0 = persist.tile([P, S0], FP8, tag="m0")
            m1 = persist.tile([P, S1], BF16, tag="m1")
            m2 = persist.tile([P, S2], BF16, tag="m2")
            m3 = persist.tile([P, S3], BF16, tag="m3")
            m4 = persist.tile([P, S4], BF16, tag="m4")
            # Stream order: s0 tiny and first (wb matmuls + the 0.7us wT exp
            # retire long before the ek chain needs ACT); kv path split over
            # both rings so proj01's gate (s1+s2) lands mid-stream; s4 last
            # (the trailing qh chain is shorter than the kv chain).
            for eng, mt, st in (
                (nc.sync, m0, s0),
                (nc.scalar, m1, s1),
                (nc.sync, m2, s2),
                (nc.scalar, m3, s3),
                (nc.sync, m4, s4),
            ):
                eng.dma_start(out=mt[:], in_=st[:])

            wbv = lambda sc: m0[:, sc * P : (sc + 1) * P]
            uTv = m0[:, TKV : TKV + R]
            kvW = lambda dc: m1[:, dc * 2 * HID : (dc + 1) * 2 * HID]
            # kv s-chunk sc, d-chunk dc: both halves half-packed (2KB lines)
            kv = lambda sc, dc: (m2 if sc < 2 else m3)[
                :, dc * 256 + (sc % 2) * P : dc * 256 + (sc % 2) * P + P
            ]
            qWT = lambda dc: m4[:, dc * HID : (dc + 1) * HID]
            qTv = lambda dc: m4[:, DC * HID + dc * R : DC * HID + (dc + 1) * R]
            qb = m4[:, S4 - 4 : S4 - 2].bitcast(F32)
            vb = m4[:, S4 - 2 : S4].bitcast(F32)

            # ---- PE warmup: the tensor engine clock ramps 1.2->2.4GHz with
            # ~3.4us of sustained work; dummy matmuls during the DMA stream
            # mean the real matmuls run at full clock ----
            # memset on DVE: gpsimd's Q7 is busy generating s0's descriptors,
            # and the warmup matmuls must start ASAP for the HAM ramp
            warm_sb = persist.tile([P, 256], BF16, tag="warm_sb")
            nc.vector.memset(warm_sb[:], 0.0)
            pwm = psum1.tile([P, 256], F32, tag="pwm")

            def fillers(n):
                # PE keeps the clock ramp through upcoming sem-wait gaps:
                # no deps, so these run while the next group's DMA lands.
                for _ in range(n):
                    nc.tensor.matmul(pwm[:, :P], lhsT=warm_sb[:, :P], rhs=warm_sb[:, :P])

            fillers(N_WARMUP)

            # ---- expwbT (s,t): all four wbias chunks in ONE PSUM bank so a
            # single ACT exp covers the whole wT; matmuls run fp8.  (start=
            # True only clears has_written bits, the data of finished chunks
            # is untouched.) ----
            wT_bf = persist.tile([P, SC, R], BF16, tag="wT_bf")
            pw = psumw.tile([P, SC, R], F32, tag="pw")
            for i in range(SC):
                nc.tensor.matmul(pw[:, i, :], lhsT=wbv(i), rhs=uTv)
            nc.scalar.activation(
                wT_bf[:], pw[:], ACT.Exp, scale=1.0 / (SWB * SWB),
            )
            fillers(FILL_WB)

            # ---- k/v projections -> ek=exp(k0), ekv=ek*v0  (s,h) ----
            ek_bf = persist.tile([P, SC, HID], BF16, tag="ek_bf")
            ekv_bf = persist.tile([P, SC, HID], BF16, tag="ekv_bf")

            def proj(pkv, i, sc):
                for dc in range(DC):
                    nc.tensor.matmul(
                        pkv[:, i, :, :],
                        lhsT=kv(sc, dc),
                        rhs=kvW(dc),
                        start=(dc == 0),
                        stop=(dc == DC - 1),
                    )

            def ekv_chunks(pkv, i, lo, n):
                # exp + v-mul over n chunks of the pair tile in one ACT/DVE op
                nc.scalar.activation(
                    ek_bf[:, lo : lo + n, :], pkv[:, i : i + n, 0, :], ACT.Exp,
                )
                nc.vector.scalar_tensor_tensor(
                    ekv_bf[:, lo : lo + n, :],
                    pkv[:, i : i + n, 1, :], 1.0,
                    ek_bf[:, lo : lo + n, :],
                    mybir.AluOpType.mult, mybir.AluOpType.mult,
                )

            # PE program order: proj half0 pair, proj half1 pair, qh, then
            # den/num ordered so only sc2/sc3's chain trails the last half.
            # Both halves use PAIRED exp/mul: one ACT + one DVE op per half
            # (the half arrives as one transfer, so there is no finer gate
            # to exploit, and pairing halves the serial ACT tail).
            pkv0 = psumk.tile([P, 2, 2, HID], F32, tag="pkv", bufs=1)
            pkv1 = psumk.tile([P, 2, 2, HID], F32, tag="pkv1", bufs=1)
            proj(pkv0, 0, 0)
            proj(pkv0, 1, 1)
            ekv_chunks(pkv0, 0, 0, 2)
            proj(pkv1, 0, 2)
            proj(pkv1, 1, 3)
            ekv_chunks(pkv1, 0, 2, 2)

            # ---- qhT (h,t); sigmoid via exp so ACT never switches tables:
            # sigmoid(qh) = 1/(1+e) with e = exp(-(qh + qW_b))  (host sends -qW_b)
            # pq/pd/pn need their OWN banks: a start=True matmul clears
            # has_written for the whole bank, so accumulation groups that
            # share a bank corrupt each other.
            pq = psum1.tile([P, R], F32, tag="pq")
            for dc in range(DC):
                nc.tensor.matmul(
                    pq[:], lhsT=qWT(dc), rhs=qTv(dc),
                    start=(dc == 0), stop=(dc == DC - 1),
                )
            e_sb = persist.tile([P, R], F32, tag="e_sb")
            nc.scalar.activation(
                e_sb[:], pq[:], ACT.Exp,
                bias=(0.0 if zero_bias else qb), scale=-1.0,
            )
            fillers(FILL_QH)

            # den/num accumulations (h,t): dens lead their nums so pd retires
            # early enough for the epilogue's t1/recip to overlap num's tail.
            pd = psum1.tile([P, R], F32, tag="pd")
            pn = psum1.tile([P, R], F32, tag="pn")

            def den(sc):
                nc.tensor.matmul(
                    pd[:], lhsT=ek_bf[:, sc, :], rhs=wT_bf[:, sc, :],
                    start=(sc == 0), stop=(sc == SC - 1),
                )

            def num(sc):
                nc.tensor.matmul(
                    pn[:], lhsT=ekv_bf[:, sc, :], rhs=wT_bf[:, sc, :],
                    start=(sc == 0), stop=(sc == SC - 1),
                )

            den(0); den(1)
            num(0); num(1)
            den(2); den(3)
            num(2); num(3)

            # ---- out = (num + vb*den) / ((1+e)*den) ----
            vbd_sb = persist.tile([P, R], F32, tag="vbd_sb")
            t1_sb = persist.tile([P, R], F32, tag="t1_sb")
            t2_sb = persist.tile([P, R], F32, tag="t2_sb")
            rec_sb = persist.tile([P, R], F32, tag="rec_sb")
            res_sb = persist.tile([P, R], BF16, tag="res_sb")
            nc.vector.scalar_tensor_tensor(
                t1_sb[:], e_sb[:], 1.0, pd[:],
                mybir.AluOpType.add, mybir.AluOpType.mult,
            )
            nc.vector.reciprocal_approx_fast(rec_sb[:], t1_sb[:])
            if zero_bias:
                # qW_b == vW_b == 0 for this problem's inputs: num needs no
                # bias term, so the chain is t1 -> recip -> mul only.
                nc.vector.tensor_mul(res_sb[:], pn[:], rec_sb[:])
            else:
                nc.scalar.mul(vbd_sb[:], pd[:], vb)
                nc.vector.tensor_add(t2_sb[:], vbd_sb[:], pn[:])
                nc.vector.tensor_mul(res_sb[:], t2_sb[:], rec_sb[:])
            # out store split across BOTH HWDGE rings: the ~0.6us issue and
            # ~0.7us first-descriptor latency are paid concurrently.
            nc.sync.dma_start(out=out[:, : R // 2], in_=res_sb[:, : R // 2])
            nc.scalar.dma_start(out=out[:, R // 2 :], in_=res_sb[:, R // 2 :])

    nc.finalize()
    return nc


_NC_CACHE = {}


def _get_nc(zero_bias=True):
    if zero_bias not in _NC_CACHE:
        _NC_CACHE[zero_bias] = _build(zero_bias)
    return _NC_CACHE[zero_bias]


def _f32_as_bf16_pair(a):
    # exact f32 bits as 2 bf16 columns (little-endian lo/hi), bitcast on device
    a = np.ascontiguousarray(np.asarray(a, np.float32).reshape(P, 1))
    return a.view(np.uint16).view(np_bf16)


def _make_in_maps(q, kv, qW_w, qW_b, kW_w, kW_b, vW_w, vW_b, w_bias_u, w_bias_v):
    f = lambda a: np.ascontiguousarray(np.asarray(a, dtype=np.float32))
    g = lambda a: np.ascontiguousarray(np.asarray(a, dtype=np.float32).astype(np_bf16))
    g8 = lambda a: np.ascontiguousarray(np.asarray(a, dtype=np.float32).astype(np_fp8))
    q, kv = f(q), f(kv)
    kvW = np.concatenate([np.asarray(kW_w), np.asarray(vW_w)], axis=0)  # (2H, DIM)
    # kvWT tiled (P, DC, 2H): [p, dc, n] = kvW[n, dc*P+p]
    kvWT_t = np.transpose(kvW.reshape(2 * HID, DC, P), (2, 1, 0))
    qWT_t = np.transpose(np.asarray(qW_w).reshape(HID, DC, P), (2, 1, 0))  # (P,DC,H)
    wbv = SWB * np.asarray(w_bias_v)  # (BDIM, TKV)
    u = SWB * np.asarray(w_bias_u)  # (TQ, BDIM)
    qf = q.reshape(B * TQ, DIM)
    # half0 half-packed [p, dc, sw(256)]; half1 quarter-packed [p, scl, dc, sw]
    halves = [
        np.transpose(kv[b].reshape(2, TKV // 2, DC, P), (3, 0, 2, 1)) for b in range(B)
    ]
    kv_s2 = [g(halves[b][:, 0].reshape(P, -1)) for b in range(B)]
    kv_s3 = [g(halves[b][:, 1].reshape(P, -1)) for b in range(B)]
    kvWT_bf = g(kvWT_t.reshape(P, -1))
    wbv_f8 = g8(wbv)
    in_maps = []
    for i in range(N_CORES):
        b = i // (N_CORES // B)
        t0 = (i % (N_CORES // B)) * R
        s0 = np.concatenate([wbv_f8, g8(u[t0 : t0 + R].T)], axis=1)  # (P, 640) fp8
        # qT tiled: [p, dc, t] = qf[i*R + t, dc*P+p]
        qT_t = np.transpose(qf[i * R : (i + 1) * R].reshape(R, DC, P), (2, 1, 0))
        nqb = _f32_as_bf16_pair(-np.asarray(qW_b, np.float32))
        vbc = _f32_as_bf16_pair(np.asarray(vW_b, np.float32))
        s4 = np.concatenate(
            [g(qWT_t.reshape(P, -1)), g(qT_t.reshape(P, -1)), nqb, vbc], axis=1
        )
        in_maps.append(
            {
                "s0": np.ascontiguousarray(s0),
                "s1": kvWT_bf,
                "s2": kv_s2[b],
                "s3": kv_s3[b],
                "s4": np.ascontiguousarray(s4),
            }
        )
    return in_maps


def _run(in_maps, trace=False, zero_bias=True):
    # The shared-pool devices occasionally throw transient
    # NRT_EXEC_UNIT_UNRECOVERABLE errors; the runtime resets the core on the
    # next open, so a short-backoff retry recovers.
    import time

    nc = _get_nc(zero_bias)
    last = None
    for attempt in range(3):
        try:
            return run_bass_kernel_spmd(
                nc, in_maps, core_ids=list(range(N_CORES)), trace=trace
            )
        except Exception as e:  # noqa: BLE001 - retry any runtime failure
            last = e
            time.sleep(2.0 * (attempt + 1))
    raise last


def kernel(**inputs) -> np.ndarray:
    zb = not (np.any(np.asarray(inputs["qW_b"])) or np.any(np.asarray(inputs["vW_b"])))
    in_maps = _make_in_maps(**inputs)
    res = _run(in_maps, zero_bias=zb)
    out = np.empty((B * TQ, HID), dtype=np.float32)
    for i in range(N_CORES):
        out[i * R : (i + 1) * R] = res.results[i]["out"].astype(np.float32).T
    return out.reshape(B, TQ, HID)


# revision 28
# speedup vs baseline: 1.0139x; 1.0139x over previous
"""AFT-Full attention on 8 TRN2 NeuronCores (Bass/Tile, no collectives).

Reference math (B=2, TQ=TKV=512, DIM=512, HID=128, BDIM=128):
    qh  = q @ qW_w.T + qW_b
    k   = kv @ kW_w.T + kW_b
    v   = kv @ vW_w.T + vW_b
    wb  = w_bias_u @ w_bias_v                       # (TQ, TKV)
    A   = exp(k[:,None] + wb[None,:,:,None])        # (B,TQ,TKV,HID)
    out = sigmoid(qh) * (sum_s A*v / sum_s A)

Factorization: exp(k + wb) = exp(k) * exp(wb) collapses the giant A
intermediate into plain matmuls:
    num[t,h] = sum_s exp(wb[t,s]) * (exp(k0[s,h]) * v0[s,h])
    den[t,h] = sum_s exp(wb[t,s]) *  exp(k0[s,h])
The k-projection bias cancels exactly in num/den; the v bias is a pure
per-h additive term:  out = sigmoid(qh) * (num0/den0 + vW_b).

Sharding: the 1024 flattened (b, t) query rows split into 8 blocks of 128 —
core i handles batch b=i//4, queries t in [128*(i%4), 128*(i%4)+128).
Each core only needs kv[b], so no collectives (their ~7us latency floor
exceeds this whole kernel).

Measured-overhead model (trace-verified): exec_time spans from the
framework's const-pool memsets to the very end of the NEFF program, which
includes a fixed ~8us postamble (254 per-semaphore zeroing instructions)
and ~1us of counted preamble -- a ~9us floor no kernel can avoid (a
2-DMA no-op kernel measures 13.7us).  The controllable body is built
around the ~1.12MB DMA stream split over BOTH HWDGE rings (each ring
issues one DMA per ~0.65us, so two rings double the front-end rate;
the 16 SDMA engines drain both round-robin, peak measured 417GB/s):

    sync:   s0[wbv|uT fp8]  s2[kv half0]  s4[qW|qT]
    scalar: s1[kvWT]        s3[kv half1]

s0 first: the wb matmuls and their 0.7us wT exp retire on ACT long
before the ek exps need the engine.  s4 last: the trailing qh chain
(4 matmuls + 1 exp) is shorter than the kv chain (proj->exp->mul->num)
that would otherwise trail.  kv halves are 2KB-line transfers; each
half's exp/mul runs PAIRED (one strided ACT op + one DVE op per half),
halving the serial ACT tail.  fp8 wb inputs: w_bias_{u,v} are ~N(0,0.02);
host pre-scales by 16 so e4m3 quantization contributes <1e-4 to wb
(CPU-verified 3.890e-3 vs 3.889e-3 all-bf16).  30 contiguous warmup
matmuls ramp the PE HAM clock (needs a fully-busy free-running ~3.4us
window) so the projection chain runs at 2.4GHz; once warm, later DMA-wait
gaps are harmless.  The output store is split across both rings so its
~0.6us issue + ~0.7us first-descriptor latency are paid concurrently.
den/num accumulation order puts pd's stop before pn's so the epilogue's
t1/recip overlap num's tail; out = (1/((1+e)*den))*num is 3 DVE ops.
Known residual: chip-level P0 power throttling (PE 2.4->2.0GHz) adds
up to ~4us on hot runs -- environmental, visible as 128ns warmup spacing
instead of 107ns in the trace.

dtype strategy: slabs are packed bf16 HOST-SIDE (wb operands fp8), halving
the DMA stream; all matmuls run native bf16/fp8, PSUM stays f32; the
result is stored/DMA'd bf16 and upcast on host (~4e-3 rel err vs the 2e-2
gate).  Full fp8 kv was tried and REJECTED: num is a random-sign sum, so
per-element ek/v quantization errors survive at full strength (4.4e-2).
Bias columns keep exact f32 bits as bf16 column pairs, bitcast back on
device.  When qW_b/vW_b are all-zero (true for this problem's
setup_inputs), a fast epilogue variant drops the bias terms; a general
variant is built lazily otherwise.

Engine choreography: sigmoid(qh) is computed as 1/(1+exp(-qh)) so ACT only
ever loads the EXP table; ek/ekv for the paired half0 use one strided ACT
exp + one DVE mul; the epilogue is 3 DVE ops (STT, fast reciprocal, mul)
with the output ring pre-warmed by a tiny primer DMA gated on ekv[sc2].
"""

import numpy as np
from ml_dtypes import bfloat16 as np_bf16
from ml_dtypes import float8_e4m3 as np_fp8

import concourse.bass as bass
import concourse.mybir as mybir
import concourse.tile as tile
from concourse import bacc
from concourse.bass_utils import run_bass_kernel_spmd

B, TQ, TKV, DIM, HID, BDIM = 2, 512, 512, 512, 128, 128
N_CORES = 8
R = (B * TQ) // N_CORES  # 128 query rows per core
P = 128
DC = DIM // P  # 4 contraction chunks for d
SC = TKV // P  # 4 contraction chunks for s
F32 = mybir.dt.float32
BF16 = mybir.dt.bfloat16
FP8 = mybir.dt.float8e4
SWB = 16.0  # host pre-scale for w_bias_u/v (fp8 range); undone in ACT scale
ACT = mybir.ActivationFunctionType
N_WARMUP = 30  # one CONTIGUOUS ~3.2us warmup run: the HAM clock unthrottles
FILL_WB = 0    # only after a fully-busy 3.4us window, so the block must not
FILL_QH = 0    # be broken by data-gated stalls; once warm, gaps are harmless

S0 = TKV + R  # 640 fp8 cols: wbv | uT
S1 = DC * 2 * HID  # 1024: kvWT
S2 = DC * (TKV // 2)  # 1024: kv s-half 0 (half layout, 2KB lines)
S3 = DC * (TKV // 2)  # 1024: kv s-half 1 (quarter layout, Q2|Q3)
S4 = DC * HID + DC * R + 4  # 1028: qWT | qT | -qb | vb


def _build(zero_bias=True):
    nc = bacc.Bacc(None)
    s0 = nc.declare_dram_parameter("s0", [P, S0], FP8, isOutput=False)
    s1 = nc.declare_dram_parameter("s1", [P, S1], BF16, isOutput=False)
    s2 = nc.declare_dram_parameter("s2", [P, S2], BF16, isOutput=False)
    s3 = nc.declare_dram_parameter("s3", [P, S3], BF16, isOutput=False)
    s4 = nc.declare_dram_parameter("s4", [P, S4], BF16, isOutput=False)
    out = nc.declare_dram_parameter("out", [HID, R], BF16, isOutput=True)

    with tile.TileContext(nc) as tc:
        with (
            tc.tile_pool(name="persist", bufs=1) as persist,
            tc.tile_pool(name="psumw", bufs=1, space="PSUM") as psumw,
            tc.tile_pool(name="psumk", bufs=2, space="PSUM") as psumk,
            tc.tile_pool(name="psum1", bufs=1, space="PSUM") as psum1,
        ):
            # ---- slab DMAs on BOTH HWDGE rings (sync + scalar): each ring
            # issues one DMA per ~0.65us, so splitting 6 transfers across two
            # rings doubles the stream's front-end issue rate.  The 16 SDMA
            # engines drain both rings round-robin per packet, so byte-fair
            # interleave preserves the completion order s0 s1 s2 Q2 Q3 s4
            # (sync carries 605KB, scalar 518KB -> s4 still lands last).
            m0 = persist.tile([P, S0], FP8, tag="m0")
            m1 = persist.tile([P, S1], BF16, tag="m1")
            m2 = persist.tile([P, S2], BF16, tag="m2")
            m3 = persist.tile([P, S3], BF16, tag="m3")
            m4 = persist.tile([P, S4], BF16, tag="m4")
            # Stream order: s0 tiny and first (wb matmuls + the 0.7us wT exp
            # retire long before the ek chain needs ACT); kv path split over
            # both rings so proj01's gate (s1+s2) lands mid-stream; s4 last
            # (the trailing qh chain is shorter than the kv chain).
            for eng, mt, st in (
                (nc.sync, m0, s0),
                (nc.scalar, m1, s1),
                (nc.sync, m2, s2),
                (nc.scalar, m3, s3),
                (nc.sync, m4, s4),
            ):
                eng.dma_start(out=mt[:], in_=st[:])

            wbv = lambda sc: m0[:, sc * P : (sc + 1) * P]
            uTv = m0[:, TKV : TKV + R]
            kvW = lambda dc: m1[:, dc * 2 * HID : (dc + 1) * 2 * HID]
            # kv s-chunk sc, d-chunk dc: both halves half-packed (2KB lines)
            kv = lambda sc, dc: (m2 if sc < 2 else m3)[
                :, dc * 256 + (sc % 2) * P : dc * 256 + (sc % 2) * P + P
            ]
            qWT = lambda dc: m4[:, dc * HID : (dc + 1) * HID]
            qTv = lambda dc: m4[:, DC * HID + dc * R : DC * HID + (dc + 1) * R]
            qb = m4[:, S4 - 4 : S4 - 2].bitcast(F32)
            vb = m4[:, S4 - 2 : S4].bitcast(F32)

            # ---- PE warmup: the tensor engine clock ramps 1.2->2.4GHz with
            # ~3.4us of sustained work; dummy matmuls during the DMA stream
            # mean the real matmuls run at full clock ----
            # memset on DVE: gpsimd's Q7 is busy generating s0's descriptors,
            # and the warmup matmuls must start ASAP for the HAM ramp
            warm_sb = persist.tile([P, 256], BF16, tag="warm_sb")
            nc.vector.memset(warm_sb[:], 0.0)
            pwm = psum1.tile([P, 256], F32, tag="pwm")

            def fillers(n):
                # PE keeps the clock ramp through upcoming sem-wait gaps:
                # no deps, so these run while the next group's DMA lands.
                for _ in range(n):
                    nc.tensor.matmul(pwm[:, :P], lhsT=warm_sb[:, :P], rhs=warm_sb[:, :P])

            fillers(N_WARMUP)

            # ---- expwbT (s,t): all four wbias chunks in ONE PSUM bank so a
            # single ACT exp covers the whole wT; matmuls run fp8.  (start=
            # True only clears has_written bits, the data of finished chunks
            # is untouched.) ----
            wT_bf = persist.tile([P, SC, R], BF16, tag="wT_bf")
            pw = psumw.tile([P, SC, R], F32, tag="pw")
            for i in range(SC):
                nc.tensor.matmul(pw[:, i, :], lhsT=wbv(i), rhs=uTv)
            nc.scalar.activation(
                wT_bf[:], pw[:], ACT.Exp, scale=1.0 / (SWB * SWB),
            )
            fillers(FILL_WB)

            # ---- k/v projections -> ek=exp(k0), ekv=ek*v0  (s,h) ----
            ek_bf = persist.tile([P, SC, HID], BF16, tag="ek_bf")
            ekv_bf = persist.tile([P, SC, HID], BF16, tag="ekv_bf")

            def proj(pkv, i, sc):
                for dc in range(DC):
                    nc.tensor.matmul(
                        pkv[:, i, :, :],
                        lhsT=kv(sc, dc),
                        rhs=kvW(dc),
                        start=(dc == 0),
                        stop=(dc == DC - 1),
                    )

            def ekv_chunks(pkv, i, lo, n):
                # exp + v-mul over n chunks of the pair tile in one ACT/DVE op
                nc.scalar.activation(
                    ek_bf[:, lo : lo + n, :], pkv[:, i : i + n, 0, :], ACT.Exp,
                )
                nc.vector.scalar_tensor_tensor(
                    ekv_bf[:, lo : lo + n, :],
                    pkv[:, i : i + n, 1, :], 1.0,
                    ek_bf[:, lo : lo + n, :],
                    mybir.AluOpType.mult, mybir.AluOpType.mult,
                )

            # PE program order: proj half0 pair, proj half1 pair, qh, then
            # den/num ordered so only sc2/sc3's chain trails the last half.
            # Both halves use PAIRED exp/mul: one ACT + one DVE op per half
            # (the half arrives as one transfer, so there is no finer gate
            # to exploit, and pairing halves the serial ACT tail).
            pkv0 = psumk.tile([P, 2, 2, HID], F32, tag="pkv", bufs=1)
            pkv1 = psumk.tile([P, 2, 2, HID], F32, tag="pkv1", bufs=1)
            proj(pkv0, 0, 0)
            proj(pkv0, 1, 1)
            ekv_chunks(pkv0, 0, 0, 2)
            proj(pkv1, 0, 2)
            proj(pkv1, 1, 3)
            ekv_chunks(pkv1, 0, 2, 2)

            # ---- qhT (h,t); sigmoid via exp so ACT never switches tables:
            # sigmoid(qh) = 1/(1+e) with e = exp(-(qh + qW_b))  (host sends -qW_b)
            # pq/pd/pn need their OWN banks: a start=True matmul clears
            # has_written for the whole bank, so accumulation groups that
            # share a bank corrupt each other.
            pq = psum1.tile([P, R], F32, tag="pq")
            for dc in range(DC):
                nc.tensor.matmul(
                    pq[:], lhsT=qWT(dc), rhs=qTv(dc),
                    start=(dc == 0), stop=(dc == DC - 1),
                )
            e_sb = persist.tile([P, R], F32, tag="e_sb")
            nc.scalar.activation(
                e_sb[:], pq[:], ACT.Exp,
                bias=(0.0 if zero_bias else qb), scale=-1.0,
            )
            fillers(FILL_QH)

            # den/num accumulations (h,t): dens lead their nums so pd retires
            # early enough for the epilogue's t1/recip to overlap num's tail.
            pd = psum1.tile([P, R], F32, tag="pd")
            pn = psum1.tile([P, R], F32, tag="pn")

            def den(sc):
                nc.tensor.matmul(
                    pd[:], lhsT=ek_bf[:, sc, :], rhs=wT_bf[:, sc, :],
                    start=(sc == 0), stop=(sc == SC - 1),
                )

            def num(sc):
                nc.tensor.matmul(
                    pn[:], lhsT=ekv_bf[:, sc, :], rhs=wT_bf[:, sc, :],
                    start=(sc == 0), stop=(sc == SC - 1),
                )

            den(0); den(1)
            num(0); num(1)
            den(2); den(3)
            num(2); num(3)

            # ---- out = (num + vb*den) / ((1+e)*den) ----
            vbd_sb = persist.tile([P, R], F32, tag="vbd_sb")
            t1_sb = persist.tile([P, R], F32, tag="t1_sb")
            t2_sb = persist.tile([P, R], F32, tag="t2_sb")
            rec_sb = persist.tile([P, R], F32, tag="rec_sb")
            res_sb = persist.tile([P, R], BF16, tag="res_sb")
            nc.vector.scalar_tensor_tensor(
                t1_sb[:], e_sb[:], 1.0, pd[:],
                mybir.AluOpType.add, mybir.AluOpType.mult,
            )
            nc.vector.reciprocal_approx_fast(rec_sb[:], t1_sb[:])
            if zero_bias:
                # qW_b == vW_b == 0 for this problem's inputs: num needs no
                # bias term, so the chain is t1 -> recip -> mul only.
                nc.vector.tensor_mul(res_sb[:], pn[:], rec_sb[:])
            else:
                nc.scalar.mul(vbd_sb[:], pd[:], vb)
                nc.vector.tensor_add(t2_sb[:], vbd_sb[:], pn[:])
                nc.vector.tensor_mul(res_sb[:], t2_sb[:], rec_sb[:])
            # out store split across BOTH HWDGE rings: the ~0.6us issue and
            # ~0.7us first-descriptor latency are paid concurrently.
            nc.sync.dma_start(out=out[:, : R // 2], in_=res_sb[:, : R // 2])
            nc.scalar.dma_start(out=out[:, R // 2 :], in_=res_sb[:, R // 2 :])

    nc.finalize()
    return nc


_NC_CACHE = {}


def _get_nc(zero_bias=True):
    if zero_bias not in _NC_CACHE:
        _NC_CACHE[zero_bias] = _build(zero_bias)
    return _NC_CACHE[zero_bias]


def _f32_as_bf16_pair(a):
    # exact f32 bits as 2 bf16 columns (little-endian lo/hi), bitcast on device
    a = np.ascontiguousarray(np.asarray(a, np.float32).reshape(P, 1))
    return a.view(np.uint16).view(np_bf16)


def _make_in_maps(q, kv, qW_w, qW_b, kW_w, kW_b, vW_w, vW_b, w_bias_u, w_bias_v):
    f = lambda a: np.ascontiguousarray(np.asarray(a, dtype=np.float32))
    g = lambda a: np.ascontiguousarray(np.asarray(a, dtype=np.float32).astype(np_bf16))
    g8 = lambda a: np.ascontiguousarray(np.asarray(a, dtype=np.float32).astype(np_fp8))
    q, kv = f(q), f(kv)
    kvW = np.concatenate([np.asarray(kW_w), np.asarray(vW_w)], axis=0)  # (2H, DIM)
    # kvWT tiled (P, DC, 2H): [p, dc, n] = kvW[n, dc*P+p]
    kvWT_t = np.transpose(kvW.reshape(2 * HID, DC, P), (2, 1, 0))
    qWT_t = np.transpose(np.asarray(qW_w).reshape(HID, DC, P), (2, 1, 0))  # (P,DC,H)
    wbv = SWB * np.asarray(w_bias_v)  # (BDIM, TKV)
    u = SWB * np.asarray(w_bias_u)  # (TQ, BDIM)
    qf = q.reshape(B * TQ, DIM)
    # half0 half-packed [p, dc, sw(256)]; half1 quarter-packed [p, scl, dc, sw]
    halves = [
        np.transpose(kv[b].reshape(2, TKV // 2, DC, P), (3, 0, 2, 1)) for b in range(B)
    ]
    kv_s2 = [g(halves[b][:, 0].reshape(P, -1)) for b in range(B)]
    kv_s3 = [g(halves[b][:, 1].reshape(P, -1)) for b in range(B)]
    kvWT_bf = g(kvWT_t.reshape(P, -1))
    wbv_f8 = g8(wbv)
    in_maps = []
    for i in range(N_CORES):
        b = i // (N_CORES // B)
        t0 = (i % (N_CORES // B)) * R
        s0 = np.concatenate([wbv_f8, g8(u[t0 : t0 + R].T)], axis=1)  # (P, 640) fp8
        # qT tiled: [p, dc, t] = qf[i*R + t, dc*P+p]
        qT_t = np.transpose(qf[i * R : (i + 1) * R].reshape(R, DC, P), (2, 1, 0))
        nqb = _f32_as_bf16_pair(-np.asarray(qW_b, np.float32))
        vbc = _f32_as_bf16_pair(np.asarray(vW_b, np.float32))
        s4 = np.concatenate(
            [g(qWT_t.reshape(P, -1)), g(qT_t.reshape(P, -1)), nqb, vbc], axis=1
        )
        in_maps.append(
            {
                "s0": np.ascontiguousarray(s0),
                "s1": kvWT_bf,
                "s2": kv_s2[b],
                "s3": kv_s3[b],
                "s4": np.ascontiguousarray(s4),
            }
        )
    return in_maps


def _run(in_maps, trace=False, zero_bias=True):
    # The shared-pool devices occasionally throw transient
    # NRT_EXEC_UNIT_UNRECOVERABLE errors; the runtime resets the core on the
    # next open, so a short-backoff retry recovers.
    import time

    nc = _get_nc(zero_bias)
    last = None
    for attempt in range(3):
        try:
            return run_bass_kernel_spmd(
                nc, in_maps, core_ids=list(range(N_CORES)), trace=trace
            )
        except Exception as e:  # noqa: BLE001 - retry any runtime failure
            last = e
            time.sleep(2.0 * (attempt + 1))
    raise last


def kernel(**inputs) -> np.ndarray:
    zb = not (np.any(np.asarray(inputs["qW_b"])) or np.any(np.asarray(inputs["vW_b"])))
    in_maps = _make_in_maps(**inputs)
    res = _run(in_maps, zero_bias=zb)
    out = np.empty((B * TQ, HID), dtype=np.float32)
    for i in range(N_CORES):
        out[i * R : (i + 1) * R] = res.results[i]["out"].astype(np.float32).T
    return out.reshape(B, TQ, HID)
